# revision 2
# baseline (speedup 1.0000x reference)
"""DeepseekV3 MLA prefill attention on 8 trn2 NeuronCores.

Strategy (single SPMD program, per-core differences live in the input data):
  Phase A: token-split A-projection, computed feature-major
           (qkv^T = W_a^T @ h^T), fused RMSNorm (partition-dim reduce via
           ones-matmul), RoPE on k_pe. gamma and the 1/sqrt(d) score scale
           are folded into the weights on the host; RoPE de-interleave is
           folded into weight column order on the host.
  AG1:     AllGather of normed latents (bf16, feature-major).
  Phase B: per-core head projections Q^T, K^T (feature-major) and V
           (token-major), heads 2c and 2c+1 on core c.
  Phase C: causal attention, S^T = K^T-tiles x Q^T-chunks, exp without
           max-subtraction (scores are O(+-8) by construction), softmax
           denominator via ones-matmul, PV accumulated feature-major,
           block-causal skipping of fully-masked tiles.
  AG2:     AllGather of attention outputs (bf16, feature-major).
  Phase E: column-split o_proj (core c computes output cols 896c..896(c+1)),
           host concatenates.
"""

import numpy as np
import ml_dtypes

T = 2048
H = 7168
NH = 16
D_NOPE = 128
D_ROPE = 64
D_V = 128
D_QK = 192
QLR = 1536
KVLR = 512
THETA = 10000.0
EPS = 1e-6
NCORES = 8
TSH = T // NCORES          # 256 tokens per core
HPC = NH // NCORES         # 2 heads per core
WO_COLS = H // NCORES      # 896 output cols per core
AGF = QLR + KVLR + D_ROPE  # 2112 gathered feature rows
NEG = -30000.0             # mask add, enough to zero bf16/f32 exp

BF16 = ml_dtypes.bfloat16

_CACHE = {}


class _Done(Exception):
    pass


def _build(upto='E'):
    import concourse.bass as bass
    import concourse.mybir as mybir
    import concourse.bacc as bacc
    import concourse.tile as tile

    dt = mybir.dt
    AF = mybir.ActivationFunctionType

    nc = bacc.Bacc(None, target_bir_lowering=False)

    # ---- per-core external inputs -------------------------------------
    hT = nc.declare_dram_parameter("hT", [H, TSH], dt.bfloat16, isOutput=False)
    wa = nc.declare_dram_parameter("wa", [17 * 56 * 128, 128], dt.bfloat16, isOutput=False)
    wqb = nc.declare_dram_parameter("wqb", [QLR, HPC * D_QK], dt.bfloat16, isOutput=False)
    wkvb = nc.declare_dram_parameter("wkvb", [KVLR, HPC * 256], dt.bfloat16, isOutput=False)
    wo = nc.declare_dram_parameter("wo", [NH * D_V, WO_COLS], dt.bfloat16, isOutput=False)
    cs_sh = nc.declare_dram_parameter("cs_sh", [64, TSH], dt.float32, isOutput=False)
    cs_full = nc.declare_dram_parameter("cs_full", [64, T], dt.float32, isOutput=False)
    masks = nc.declare_dram_parameter("masks", [4 * 128, 512], dt.float32, isOutput=False)
    out = nc.declare_dram_parameter("out", [WO_COLS, T], dt.float32, isOutput=True)

    rg = [list(range(NCORES))]

    _build_body(nc, mybir, upto, hT, wa, wqb, wkvb, wo, cs_sh, cs_full,
                masks, out)
    nc.compile()
    return nc


def _build_body(nc, mybir, upto, hT, wa, wqb, wkvb, wo, cs_sh, cs_full,
                masks, out):
    import concourse.tile as tile
    dt = mybir.dt
    AF = mybir.ActivationFunctionType
    rg = [list(range(NCORES))]

    with tile.TileContext(nc) as tc:
        import contextlib

        top = contextlib.ExitStack()
        with top:
            const = top.enter_context(tc.tile_pool(name="const", bufs=1))
            wpool = top.enter_context(tc.tile_pool(name="wpool", bufs=1))
            dram = top.enter_context(tc.tile_pool(name="dram", bufs=1, space="DRAM"))

            ones_b = const.tile([128, 1], dt.bfloat16, tag="ones_b", name="ones_b")
            nc.vector.memset(ones_b[:], 1.0)
            ones_f = const.tile([1, 128], dt.float32, tag="ones_f", name="ones_f")
            nc.vector.memset(ones_f[:], 1.0)
            mask_sb = const.tile([128, 4, 512], dt.float32, tag="mask", name="mask")
            for i in range(4):
                nc.sync.dma_start(mask_sb[:, i, :], masks[i * 128:(i + 1) * 128, :])
            csc_f = const.tile([32, T], dt.float32, tag="csc_f", name="csc_f")
            nc.sync.dma_start(csc_f[:], cs_full[0:32, :])
            csn_f = const.tile([32, T], dt.float32, tag="csn_f", name="csn_f")
            nc.sync.dma_start(csn_f[:], cs_full[32:64, :])
            csc_s = const.tile([32, TSH], dt.float32, tag="csc_s", name="csc_s")
            nc.sync.dma_start(csc_s[:], cs_sh[0:32, :])
            csn_s = const.tile([32, TSH], dt.float32, tag="csn_s", name="csn_s")
            nc.sync.dma_start(csn_s[:], cs_sh[32:64, :])

            # resident weights for phases B and E
            wqb_t = wpool.tile([128, 12, HPC * D_QK], dt.bfloat16, tag="wqb", name="wqb")
            for a0 in range(0, 12, 4):
                nc.sync.dma_start(
                    wqb_t[:, a0:a0 + 4, :],
                    wqb[a0 * 128:(a0 + 4) * 128, :].rearrange(
                        "(a p) f -> p a f", p=128))
            wkvb_t = wpool.tile([128, 4, HPC * 256], dt.bfloat16, tag="wkvb", name="wkvb")
            nc.sync.dma_start(
                wkvb_t[:], wkvb.rearrange("(a p) f -> p a f", p=128))
            wo_t = wpool.tile([128, 16, WO_COLS], dt.bfloat16, tag="wo", name="wo")
            for a0 in range(0, 16, 4):
                nc.sync.dma_start(
                    wo_t[:, a0:a0 + 4, :],
                    wo[a0 * 128:(a0 + 4) * 128, :].rearrange(
                        "(a p) f -> p a f", p=128))

            # collective buffers (AG1 split: kv+k_pe gathered early, q late;
            # AG2 split per head so it overlaps the other head's attention)
            ag1a_in = dram.tile([576, TSH], dt.bfloat16, tag="ag1ai", name="ag1ai")
            ag1a_out = dram.tile([NCORES * 576, TSH], dt.bfloat16, tag="ag1ao", name="ag1ao", addr_space="Shared")
            ag1b_in = dram.tile([QLR, TSH], dt.bfloat16, tag="ag1bi", name="ag1bi")
            ag1b_out = dram.tile([NCORES * QLR, TSH], dt.bfloat16, tag="ag1bo", name="ag1bo", addr_space="Shared")
            ag2_in = [dram.tile([D_V, T], dt.bfloat16, tag=f"ag2i{h}", name=f"ag2i{h}")
                      for h in range(HPC)]
            ag2_out = [dram.tile([NCORES * D_V, T], dt.bfloat16, tag=f"ag2o{h}",
                                 name=f"ag2o{h}", addr_space="Shared")
                       for h in range(HPC)]

            # ============================================================
            # Phase A: qkv^T = Wa^T @ h^T   [2112, 256] feature-major
            # ============================================================
            with contextlib.ExitStack() as pa:
                h_pool = pa.enter_context(tc.tile_pool(name="h", bufs=1))
                wa_pool = pa.enter_context(tc.tile_pool(name="wa", bufs=6))
                qkv_pool = pa.enter_context(tc.tile_pool(name="qkv", bufs=1))
                x2_pool = pa.enter_context(tc.tile_pool(name="x2", bufs=3))
                agt_pool = pa.enter_context(tc.tile_pool(name="agt", bufs=3))
                ps_a = pa.enter_context(tc.tile_pool(name="ps_a", bufs=3, space="PSUM"))
                ps_ss = pa.enter_context(tc.tile_pool(name="ps_ss", bufs=1, space="PSUM"))
                ps_bc = pa.enter_context(tc.tile_pool(name="ps_bc", bufs=1, space="PSUM"))

                h_all = h_pool.tile([128, 56, TSH], dt.bfloat16, tag="h_all", name="h_all")
                for a0 in range(0, 56, 7):
                    nc.sync.dma_start(
                        h_all[:, a0:a0 + 7, :],
                        hT[a0 * 128:(a0 + 7) * 128, :].rearrange(
                            "(a p) t -> p a t", p=128))

                qkv = [
                    qkv_pool.tile([128, TSH], dt.float32, tag=f"qkv{m}", name=f"qkv{m}")
                    for m in range(16)
                ]
                kp_raw = qkv_pool.tile([64, TSH], dt.float32, tag="kp_raw", name="kp_raw")
                kp2 = qkv_pool.tile([32, TSH], dt.float32, tag="kp2", name="kp2")

                ss_q = ps_ss.tile([1, TSH], dt.float32, tag="ssq", name="ssq")
                ss_kv = ps_ss.tile([1, TSH], dt.float32, tag="sskv", name="sskv")

                def rstd_bcast(ss, d, name):
                    ms = x2_pool.tile([1, TSH], dt.float32, tag="ms", name="ms")
                    nc.scalar.activation(ms[:], ss[:], AF.Copy, bias=EPS, scale=1.0 / d)
                    inv = x2_pool.tile([1, TSH], dt.float32, tag="inv", name="inv")
                    nc.vector.reciprocal(inv[:], ms[:])
                    rstd = x2_pool.tile([1, TSH], dt.float32, tag="rstd", name="rstd")
                    nc.scalar.activation(rstd[:], inv[:], AF.Sqrt)
                    pb = ps_bc.tile([128, TSH], dt.float32, tag=f"bc{name}", name=f"bc{name}")
                    nc.tensor.matmul(pb[:], ones_f[:], rstd[:], start=True, stop=True)
                    return pb

                for m in [12, 13, 14, 15, 16] + list(range(12)):
                    mp = 64 if m == 16 else 128
                    psum = ps_a.tile([128, TSH], dt.float32, tag="pa", name="pa")
                    for kc in range(7):
                        chunk = wa_pool.tile([128, 8, 128], dt.bfloat16, tag="wa_c", name="wa_c")
                        r0 = (m * 56 + kc * 8) * 128
                        nc.sync.dma_start(
                            chunk[:],
                            wa[r0:r0 + 1024, :].rearrange("(p a) f -> p a f", a=8),
                        )
                        for k8 in range(8):
                            k = kc * 8 + k8
                            nc.tensor.matmul(
                                psum[:mp, :],
                                chunk[:, k8, :mp],
                                h_all[:, k, :],
                                start=(k == 0),
                                stop=(k == 55),
                                skip_group_check=True,
                            )
                    # evict to f32 SBUF
                    if m < 16:
                        nc.scalar.copy(qkv[m][:], psum[:])
                    else:
                        nc.scalar.copy(kp_raw[:], psum[:64, :])
                        # move the x2 half to base partition 0 for the DVE ops
                        nc.sync.dma_start(kp2[:], kp_raw[32:64, :])
                    if m < 16:
                        # squared tile for the RMS partition-sum
                        x2 = x2_pool.tile([128, TSH], dt.bfloat16, tag="x2", name="x2")
                        nc.vector.tensor_mul(x2[:], qkv[m][:], qkv[m][:])
                        ss = ss_q if m < 12 else ss_kv
                        first = (m == 0) or (m == 12)
                        last = (m == 11) or (m == 15)
                        nc.tensor.matmul(
                            ss[:], ones_b[:], x2[:], start=first, stop=last,
                            skip_group_check=True,
                        )
                    if m == 16:
                        # kv group + k_pe done: norm kv, rope k_pe, launch AG1a
                        bc_kv = rstd_bcast(ss_kv, KVLR, "kv")
                        for mm in range(12, 16):
                            agt = agt_pool.tile([128, TSH], dt.bfloat16, tag="agt", name="agt")
                            nc.vector.tensor_mul(agt[:], qkv[mm][:], bc_kv[:])
                            nc.sync.dma_start(
                                ag1a_in[(mm - 12) * 128:(mm - 11) * 128, :], agt[:])
                        kr1 = agt_pool.tile([32, TSH], dt.bfloat16, tag="kr1", name="kr1")
                        kr2 = agt_pool.tile([32, TSH], dt.bfloat16, tag="kr2", name="kr2")
                        t1 = x2_pool.tile([32, TSH], dt.bfloat16, tag="t1", name="t1")
                        t2 = x2_pool.tile([32, TSH], dt.bfloat16, tag="t2", name="t2")
                        nc.vector.tensor_mul(t1[:], kp_raw[0:32, :], csc_s[:])
                        nc.vector.tensor_mul(t2[:], kp2[:], csn_s[:])
                        nc.vector.tensor_sub(kr1[:], t1[:], t2[:])
                        t3 = x2_pool.tile([32, TSH], dt.bfloat16, tag="t1", name="t1")
                        t4 = x2_pool.tile([32, TSH], dt.bfloat16, tag="t2", name="t2")
                        nc.vector.tensor_mul(t3[:], kp_raw[0:32, :], csn_s[:])
                        nc.vector.tensor_mul(t4[:], kp2[:], csc_s[:])
                        nc.vector.tensor_add(kr2[:], t3[:], t4[:])
                        nc.sync.dma_start(ag1a_in[512:544, :], kr1[:])
                        nc.sync.dma_start(ag1a_in[544:576, :], kr2[:])
                        nc.gpsimd.collective_compute(
                            "AllGather", mybir.AluOpType.bypass,
                            replica_groups=rg,
                            ins=[ag1a_in.opt()], outs=[ag1a_out.opt()])

                # q group done: norm q, launch AG1b
                bc_q = rstd_bcast(ss_q, QLR, "q")
                for m in range(12):
                    agt = agt_pool.tile([128, TSH], dt.bfloat16, tag="agt", name="agt")
                    nc.vector.tensor_mul(agt[:], qkv[m][:], bc_q[:])
                    nc.sync.dma_start(ag1b_in[m * 128:(m + 1) * 128, :], agt[:])

            # ============================================================
            # AG1b
            # ============================================================
            if upto == 'A':
                return
            nc.gpsimd.collective_compute(
                "AllGather",
                mybir.AluOpType.bypass,
                replica_groups=rg,
                ins=[ag1b_in.opt()],
                outs=[ag1b_out.opt()],
            )

            # ============================================================
            # Phase B: Q^T, K^T (feature-major) and V (token-major)
            # ============================================================
            bpools = contextlib.ExitStack()
            with bpools:
                act = bpools.enter_context(tc.tile_pool(name="act", bufs=1))
                agq_pool = bpools.enter_context(tc.tile_pool(name="agq", bufs=3))
                agkv_pool = bpools.enter_context(tc.tile_pool(name="agkv", bufs=2))
                pbp = contextlib.ExitStack()
                ps_q = pbp.enter_context(tc.tile_pool(name="ps_q", bufs=4, space="PSUM"))
                ps_kv = pbp.enter_context(tc.tile_pool(name="ps_kv", bufs=2, space="PSUM"))

                qt_n = [act.tile([128, T], dt.bfloat16, tag=f"qtn{h}", name=f"qtn{h}") for h in range(HPC)]
                qrw = [act.tile([64, T], dt.bfloat16, tag=f"qrw{h}", name=f"qrw{h}") for h in range(HPC)]
                qrr2 = [act.tile([32, T], dt.bfloat16, tag=f"qrr2{h}", name=f"qrr2{h}") for h in range(HPC)]
                qt_r = [act.tile([64, T], dt.bfloat16, tag=f"qtr{h}", name=f"qtr{h}") for h in range(HPC)]
                kt_n = [act.tile([128, T], dt.bfloat16, tag=f"ktn{h}", name=f"ktn{h}") for h in range(HPC)]
                kpe_t = act.tile([64, T], dt.bfloat16, tag="kpet", name="kpet")
                # v2_t[ti]: both heads' V for token tile ti, cols h*128..
                v2_t = [act.tile([128, 2 * D_V], dt.bfloat16, tag=f"v{i}", name=f"v{i}")
                        for i in range(16)]

                for rp in range(4):
                    tsl = slice(rp * 512, (rp + 1) * 512)
                    # --- Q path (two ranks per 512-wide chunk) ---
                    pq = []
                    for h in range(HPC):
                        pn = ps_q.tile([128, 512], dt.float32, tag="pq", name="pq")
                        pr = ps_q.tile([64, 512], dt.float32, tag="pq", name="pq")
                        pq.append((pn, pr))
                    for kq in range(12):
                        aq = agq_pool.tile([128, 512], dt.bfloat16, tag="aq", name="aq")
                        for s in range(2):
                            r = 2 * rp + s
                            nc.sync.dma_start(
                                aq[:, s * TSH:(s + 1) * TSH],
                                ag1b_out[r * QLR + kq * 128: r * QLR + (kq + 1) * 128, :])
                        for h in range(HPC):
                            pn, pr = pq[h]
                            c0 = h * D_QK
                            nc.tensor.matmul(
                                pn[:], wqb_t[:, kq, c0:c0 + 128], aq[:],
                                start=(kq == 0), stop=(kq == 11),
                                skip_group_check=True)
                            nc.tensor.matmul(
                                pr[:64, :], wqb_t[:, kq, c0 + 128:c0 + 192], aq[:],
                                start=(kq == 0), stop=(kq == 11),
                                skip_group_check=True)
                    for h in range(HPC):
                        pn, pr = pq[h]
                        nc.scalar.copy(qt_n[h][:, tsl], pn[:])
                        nc.scalar.copy(qrw[h][:, tsl], pr[:64, :])
                    # --- KV path ---
                    akv = [agkv_pool.tile([128, 512], dt.bfloat16, tag=f"akv{kk}", name=f"akv{kk}")
                           for kk in range(4)]
                    for kk in range(4):
                        for s in range(2):
                            r = 2 * rp + s
                            nc.sync.dma_start(
                                akv[kk][:, s * TSH:(s + 1) * TSH],
                                ag1a_out[r * 576 + kk * 128: r * 576 + (kk + 1) * 128, :])
                    for h in range(HPC):
                        pk = ps_kv.tile([128, 512], dt.float32, tag="pkv", name="pkv")
                        for kk in range(4):
                            nc.tensor.matmul(
                                pk[:], wkvb_t[:, kk, h * 128:(h + 1) * 128], akv[kk][:],
                                start=(kk == 0), stop=(kk == 3),
                                skip_group_check=True)
                        nc.scalar.copy(kt_n[h][:, tsl], pk[:])
                    # --- V for both heads at once (wkvb cols 256..512) ---
                    for s4 in range(4):
                        ti = 4 * rp + s4
                        pv = ps_kv.tile([128, 2 * D_V], dt.float32, tag="pv_b", name="pv_b")
                        for kk in range(4):
                            nc.tensor.matmul(
                                pv[:],
                                akv[kk][:, s4 * 128:(s4 + 1) * 128],
                                wkvb_t[:, kk, 256:512],
                                start=(kk == 0), stop=(kk == 3),
                                skip_group_check=True)
                        nc.scalar.copy(v2_t[ti][:], pv[:])
                    # --- shared roped k_pe ---
                    for s in range(2):
                        r = 2 * rp + s
                        nc.sync.dma_start(
                            kpe_t[:, r * TSH:(r + 1) * TSH],
                            ag1a_out[r * 576 + 512: r * 576 + 576, :])

                pbp.close()

                # RoPE on q (full width); all elementwise ops at base
                # partition 0, the upper half moves via SBUF->SBUF DMA.
                for h in range(HPC):
                    nc.sync.dma_start(qrr2_ := qrr2[h][:], qrw[h][32:64, :])
                    x1 = qrw[h][0:32, :]
                    x2 = qrr2_
                    t1 = agq_pool.tile([32, T], dt.bfloat16, tag="qt1", name="qt1")
                    t2 = agq_pool.tile([32, T], dt.bfloat16, tag="qt2", name="qt2")
                    nc.vector.tensor_mul(t1[:], x1, csc_f[:])
                    nc.vector.tensor_mul(t2[:], x2, csn_f[:])
                    nc.vector.tensor_sub(qt_r[h][0:32, :], t1[:], t2[:])
                    t3 = agq_pool.tile([32, T], dt.bfloat16, tag="qt1", name="qt1")
                    t4 = agq_pool.tile([32, T], dt.bfloat16, tag="qt2", name="qt2")
                    nc.vector.tensor_mul(t3[:], x1, csn_f[:])
                    nc.vector.tensor_mul(t4[:], x2, csc_f[:])
                    r2t = agq_pool.tile([32, T], dt.bfloat16, tag="r2t", name="r2t")
                    nc.vector.tensor_add(r2t[:], t3[:], t4[:])
                    nc.sync.dma_start(qt_r[h][32:64, :], r2t[:])

                if upto == 'B':
                    return
                # ========================================================
                # Phase C: attention per head
                # ========================================================
                with contextlib.ExitStack() as pc:
                    pt_pool = pc.enter_context(tc.tile_pool(name="pt", bufs=3))
                    sm_pool = pc.enter_context(tc.tile_pool(name="sm", bufs=2))
                    ps_s = pc.enter_context(tc.tile_pool(name="ps_s", bufs=3, space="PSUM"))
                    ps_pv = pc.enter_context(tc.tile_pool(name="ps_pv", bufs=2, space="PSUM"))
                    ps_l = pc.enter_context(tc.tile_pool(name="ps_l", bufs=1, space="PSUM"))
                    ps_b = pc.enter_context(tc.tile_pool(name="ps_b", bufs=1, space="PSUM"))

                    for h in range(HPC):
                        attn = act.tile([128, T], dt.bfloat16, tag=f"attn{h}", name=f"attn{h}")
                        for j in range(4):
                            qsl = slice(j * 512, (j + 1) * 512)
                            nk = 4 * j + 4
                            ppv = ps_pv.tile([128, 512], dt.float32, tag="ppv", name="ppv")
                            pl = ps_l.tile([1, 512], dt.float32, tag="pl", name="pl")
                            for ki in range(nk):
                                ksl = slice(ki * 128, (ki + 1) * 128)
                                # diagonal tiles: columns left of the diagonal
                                # are fully masked -> compute only the suffix
                                off = max(0, (ki - 4 * j) * 128)
                                w = 512 - off
                                qs0 = j * 512 + off
                                ps = ps_s.tile([128, 512], dt.float32, tag="ps", name="ps")
                                nc.tensor.matmul(
                                    ps[:, off:], kt_n[h][:, ksl],
                                    qt_n[h][:, qs0:qs0 + w],
                                    start=True, stop=False, skip_group_check=True)
                                nc.tensor.matmul(
                                    ps[:, off:], kpe_t[:, ksl],
                                    qt_r[h][:, qs0:qs0 + w],
                                    start=False, stop=True, skip_group_check=True)
                                if ki >= 4 * j:
                                    nc.vector.tensor_add(
                                        ps[:, off:], ps[:, off:], mask_sb[:, 0, :w])
                                pt = pt_pool.tile([128, 512], dt.bfloat16, tag="pt", name="pt")
                                nc.scalar.activation(pt[:, off:], ps[:, off:], AF.Exp)
                                nc.tensor.matmul(
                                    pl[:, off:], ones_b[:], pt[:, off:],
                                    start=(ki == 0), stop=(ki == nk - 1),
                                    skip_group_check=True)
                                nc.tensor.matmul(
                                    ppv[:, off:], v2_t[ki][:, h * D_V:(h + 1) * D_V],
                                    pt[:, off:],
                                    start=(ki == 0), stop=(ki == nk - 1),
                                    skip_group_check=True)
                            # normalize: attn^T[:, qsl] = ppv * (1/l) broadcast
                            rl = sm_pool.tile([1, 512], dt.float32, tag="rl", name="rl")
                            nc.vector.reciprocal(rl[:], pl[:])
                            pb = ps_b.tile([128, 512], dt.float32, tag="pb", name="pb")
                            nc.tensor.matmul(pb[:], ones_f[:], rl[:],
                                             start=True, stop=True,
                                             skip_group_check=True)
                            rb = sm_pool.tile([128, 512], dt.float32, tag="rb", name="rb")
                            nc.vector.tensor_copy(rb[:], pb[:])
                            nc.vector.tensor_mul(attn[:, qsl], ppv[:], rb[:])
                        nc.sync.dma_start(ag2_in[h][:], attn[:])
                        if upto != 'C':
                            nc.gpsimd.collective_compute(
                                "AllGather", mybir.AluOpType.bypass,
                                replica_groups=rg,
                                ins=[ag2_in[h].opt()], outs=[ag2_out[h].opt()])

            if upto == 'C':
                return

            # ============================================================
            # Phase E: o_proj column slice: out = attn_full^T.T @ wo_cols
            # ============================================================
            with contextlib.ExitStack() as pe:
                ao_pool = pe.enter_context(tc.tile_pool(name="ao", bufs=1))
                oo_pool = pe.enter_context(tc.tile_pool(name="oo", bufs=3))
                ps_o = pe.enter_context(tc.tile_pool(name="ps_o", bufs=5, space="PSUM"))

                strips = [ao_pool.tile([128, T], dt.bfloat16, tag=f"st{kf}", name=f"st{kf}")
                          for kf in range(16)]
                for kf in range(16):
                    srcb = ag2_out[kf % 2]
                    rr = kf // 2
                    nc.sync.dma_start(
                        strips[kf][:], srcb[rr * 128:(rr + 1) * 128, :])

                for mt in range(7):
                    msl = slice(mt * 128, (mt + 1) * 128)
                    pes = [ps_o.tile([128, 512], dt.float32, tag="po", name="po")
                           for _ in range(4)]
                    for kf in range(16):
                        for n in range(4):
                            nc.tensor.matmul(
                                pes[n][:], wo_t[:, kf, msl],
                                strips[kf][:, n * 512:(n + 1) * 512],
                                start=(kf == 0), stop=(kf == 15),
                                skip_group_check=True)
                    ot = oo_pool.tile([128, T], dt.float32, tag="ot", name="ot")
                    for n in range(4):
                        nc.scalar.copy(ot[:, n * 512:(n + 1) * 512], pes[n][:])
                    nc.sync.dma_start(out[msl, :], ot[:])


def _prep_inputs(hidden_states, positions, W_qkv_a, gamma_q, W_qb, gamma_kv,
                 W_kvb, W_o):
    f32 = np.float32
    perm = np.concatenate([np.arange(0, D_ROPE, 2), np.arange(1, D_ROPE, 2)])
    scale = np.float32(D_QK ** -0.5)

    # A-projection weights: de-interleave k_pe output cols, block layout
    Wa = np.asarray(W_qkv_a, f32).copy()
    Wa[:, QLR + KVLR:] = Wa[:, QLR + KVLR:][:, perm]
    Wa = np.concatenate([Wa, np.zeros((H, 64), f32)], axis=1)  # pad 2112->2176
    # chunk (m, kc) stored so each SBUF partition line is 2KB contiguous:
    # rows (m*56 + kc*8)*128 + p*8 + k8, cols f
    wa_b = (
        Wa.reshape(7, 8, 128, 17, 128)   # [kc, k8, p, m, f]
        .transpose(3, 0, 2, 1, 4)        # [m, kc, p, k8, f]
        .reshape(17 * 56 * 128, 128)
        .astype(BF16)
    )

    # q_b weights: fold gamma_q and score scale, de-interleave rope cols
    Wqb = (np.asarray(W_qb, f32) * np.asarray(gamma_q, f32)[:, None] * scale)
    Wqb = Wqb.reshape(QLR, NH, D_QK)
    Wqb = np.concatenate([Wqb[:, :, :D_NOPE], Wqb[:, :, D_NOPE:][:, :, perm]], axis=2)

    # kv_b weights: fold gamma_kv
    Wkvb = (np.asarray(W_kvb, f32) * np.asarray(gamma_kv, f32)[:, None])
    Wkvb = Wkvb.reshape(KVLR, NH, D_NOPE + D_V)

    Wo = np.asarray(W_o, f32)

    hTf = np.asarray(hidden_states, f32).T.astype(BF16)  # [H, T]

    pos = np.asarray(positions, f32)
    inv_freq = 1.0 / (THETA ** (np.arange(D_ROPE // 2, dtype=f32) / (D_ROPE // 2)))
    freqs = pos[:, None] * inv_freq[None, :]          # [T, 32]
    cos = np.cos(freqs).astype(f32).T                 # [32, T]
    sin = np.sin(freqs).astype(f32).T
    cs = np.concatenate([cos, sin], axis=0)           # [64, T]

    m = np.zeros((4, 128, 512), f32)
    kk = np.arange(128)[:, None]
    qq = np.arange(512)[None, :]
    for oi in range(4):
        m[oi][qq < kk + 128 * oi] = NEG
    masks = m.reshape(4 * 128, 512)

    in_maps = []
    for c in range(NCORES):
        hds = slice(2 * c, 2 * c + 2)
        in_maps.append({
            "hT": np.ascontiguousarray(hTf[:, c * TSH:(c + 1) * TSH]),
            "wa": wa_b,
            "wqb": np.ascontiguousarray(
                Wqb[:, hds, :].reshape(QLR, HPC * D_QK)).astype(BF16),
            "wkvb": np.ascontiguousarray(np.concatenate(
                [Wkvb[:, 2 * c, :D_NOPE], Wkvb[:, 2 * c + 1, :D_NOPE],
                 Wkvb[:, 2 * c, D_NOPE:], Wkvb[:, 2 * c + 1, D_NOPE:]],
                axis=1)).astype(BF16),
            "wo": np.ascontiguousarray(
                Wo[:, c * WO_COLS:(c + 1) * WO_COLS]).astype(BF16),
            "cs_sh": np.ascontiguousarray(cs[:, c * TSH:(c + 1) * TSH]),
            "cs_full": cs,
            "masks": masks,
        })
    return in_maps


def kernel(hidden_states, positions, W_qkv_a, gamma_q, W_qb, gamma_kv, W_kvb,
           W_o, _trace=False, _tmpdir=None):
    from concourse.bass_utils import run_bass_kernel_spmd

    if "nc" not in _CACHE:
        _CACHE["nc"] = _build()
    nc = _CACHE["nc"]

    in_maps = _prep_inputs(hidden_states, positions, W_qkv_a, gamma_q, W_qb,
                           gamma_kv, W_kvb, W_o)
    res = run_bass_kernel_spmd(nc, in_maps, list(range(NCORES)), trace=_trace,
                               tmpdir=_tmpdir)
    _CACHE["last_result"] = res
    out = np.concatenate(
        [res.results[c]["out"].T for c in range(NCORES)], axis=1)
    return out.astype(np.float32)



# revision 5
# speedup vs baseline: 1.2424x; 1.2424x over previous
"""DeepseekV3 MLA prefill attention on 8 trn2 NeuronCores.

Strategy (single SPMD program, per-core differences live in the input data):
  Phase A: token-split A-projection, computed feature-major
           (qkv^T = W_a^T @ h^T), fused RMSNorm (partition-dim reduce via
           ones-matmul), RoPE on k_pe. gamma and the 1/sqrt(d) score scale
           are folded into the weights on the host; RoPE de-interleave is
           folded into weight column order on the host.
           q-latents are gathered RAW (norm applied in phase B via a
           gathered rstd row) so the q AllGather can be split into three
           parts that launch as the m-loop progresses.
  AGs:     kpe AG + kv AG launched mid-A; 3 q-latent AG parts; all bf16,
           p-major blocked layouts so phase-B loads are contiguous.
  Phase B: per-core head projections Q^T, K^T (feature-major) and V
           (token-major), heads 2c and 2c+1 on core c. KV/V work emitted
           before Q work so the PE has work while q AG parts land.
           Packed Q weights: 3 matmuls per (rp, kq) instead of 4.
           RoPE on q done per-rp.
  Phase C: causal attention, S^T = K^T-tiles x Q^T-chunks, exp without
           max-subtraction, softmax denominator via ones-matmul,
           broadcast-first reciprocal, PV accumulated feature-major,
           block-causal skipping of fully-masked tiles.
  AG2:     AllGather of attention outputs per head (bf16, feature-major).
  Phase E: column-split o_proj in two K-waves (head-0 strips consumed
           while head-1's AllGather is in flight; bf16 partials + DVE add).
"""

import numpy as np
import ml_dtypes

T = 2048
H = 7168
NH = 16
D_NOPE = 128
D_ROPE = 64
D_V = 128
D_QK = 192
QLR = 1536
KVLR = 512
THETA = 10000.0
EPS = 1e-6
NCORES = 8
TSH = T // NCORES          # 256 tokens per core
HPC = NH // NCORES         # 2 heads per core
WO_COLS = H // NCORES      # 896 output cols per core
NEG = -30000.0             # mask add, enough to zero bf16/f32 exp

BF16 = ml_dtypes.bfloat16

_CACHE = {}


def _build():
    import concourse.mybir as mybir
    import concourse.bacc as bacc

    dt = mybir.dt

    nc = bacc.Bacc(None, target_bir_lowering=False)

    # ---- per-core external inputs (all pre-blocked p-major on host) ----
    hT = nc.declare_dram_parameter("hT", [128, 56 * TSH], dt.bfloat16, isOutput=False)
    wa = nc.declare_dram_parameter("wa", [17 * 56 * 128, 128], dt.bfloat16, isOutput=False)
    wqb = nc.declare_dram_parameter("wqb", [128, 12 * 384], dt.bfloat16, isOutput=False)
    wkvb = nc.declare_dram_parameter("wkvb", [128, 4 * 512], dt.bfloat16, isOutput=False)
    wo = nc.declare_dram_parameter("wo", [128, 16 * WO_COLS], dt.bfloat16, isOutput=False)
    cs_sh = nc.declare_dram_parameter("cs_sh", [64, TSH], dt.float32, isOutput=False)
    cs_full = nc.declare_dram_parameter("cs_full", [64, T], dt.float32, isOutput=False)
    mask1 = nc.declare_dram_parameter("mask1", [128, 128], dt.float32, isOutput=False)
    out = nc.declare_dram_parameter("out", [WO_COLS, T], dt.float32, isOutput=True)

    _build_body(nc, mybir, hT, wa, wqb, wkvb, wo, cs_sh, cs_full, mask1, out)
    nc.compile()
    return nc


def _build_body(nc, mybir, hT, wa, wqb, wkvb, wo, cs_sh, cs_full, mask1, out):
    import concourse.tile as tile
    import contextlib
    dt = mybir.dt
    AF = mybir.ActivationFunctionType
    rg = [list(range(NCORES))]

    with tile.TileContext(nc) as tc:
        top = contextlib.ExitStack()
        with top:
            const = top.enter_context(tc.tile_pool(name="const", bufs=1))
            wpool = top.enter_context(tc.tile_pool(name="wpool", bufs=1))
            dram = top.enter_context(tc.tile_pool(name="dram", bufs=1, space="DRAM"))

            ones_b = const.tile([128, 1], dt.bfloat16, tag="ones_b", name="ones_b")
            nc.vector.memset(ones_b[:], 1.0)
            ones_f = const.tile([1, 128], dt.float32, tag="ones_f", name="ones_f")
            nc.vector.memset(ones_f[:], 1.0)
            mask_sb = const.tile([128, 128], dt.float32, tag="mask", name="mask")
            nc.sync.dma_start(mask_sb[:], mask1[:, :])
            csc_f = const.tile([32, T], dt.float32, tag="csc_f", name="csc_f")
            nc.sync.dma_start(csc_f[:], cs_full[0:32, :])
            csn_f = const.tile([32, T], dt.float32, tag="csn_f", name="csn_f")
            nc.sync.dma_start(csn_f[:], cs_full[32:64, :])
            csc_s = const.tile([32, TSH], dt.float32, tag="csc_s", name="csc_s")
            nc.sync.dma_start(csc_s[:], cs_sh[0:32, :])
            csn_s = const.tile([32, TSH], dt.float32, tag="csn_s", name="csn_s")
            nc.sync.dma_start(csn_s[:], cs_sh[32:64, :])

            # resident weights for phases B and E (single contiguous DMAs)
            wqb_t = wpool.tile([128, 12, 384], dt.bfloat16, tag="wqb", name="wqb")
            nc.sync.dma_start(wqb_t[:], wqb.rearrange("p (a f) -> p a f", a=12))
            wkvb_t = wpool.tile([128, 4, 512], dt.bfloat16, tag="wkvb", name="wkvb")
            nc.sync.dma_start(wkvb_t[:], wkvb.rearrange("p (a f) -> p a f", a=4))
            wo_t = wpool.tile([128, 16, WO_COLS], dt.bfloat16, tag="wo", name="wo")
            nc.sync.dma_start(wo_t[:], wo.rearrange("p (a f) -> p a f", a=16))

            # collective buffers, all p-major blocked
            agkpe_in = dram.tile([128, 128], dt.bfloat16, tag="agkpei", name="agkpei")
            agkpe_out = dram.tile([NCORES * 128, 128], dt.bfloat16, tag="agkpeo",
                                  name="agkpeo", addr_space="Shared")
            agkv_in = dram.tile([128, 4, TSH], dt.bfloat16, tag="agkvi", name="agkvi")
            agkv_out = dram.tile([NCORES * 128, 4, TSH], dt.bfloat16, tag="agkvo",
                                 name="agkvo", addr_space="Shared")
            # q parts: 0 -> m 0..3, 1 -> m 4..7, 2 -> m 8..11 + rstd row
            agq_in = [dram.tile([128, 4, TSH], dt.bfloat16, tag=f"agqi{p}", name=f"agqi{p}")
                      for p in range(2)]
            agq_in.append(dram.tile([128, 5, TSH], dt.bfloat16, tag="agqi2", name="agqi2"))
            agq_out = [dram.tile([NCORES * 128, 4, TSH], dt.bfloat16, tag=f"agqo{p}",
                                 name=f"agqo{p}", addr_space="Shared") for p in range(2)]
            agq_out.append(dram.tile([NCORES * 128, 5, TSH], dt.bfloat16, tag="agqo2",
                                     name="agqo2", addr_space="Shared"))
            ag2_in = [dram.tile([D_V, T], dt.bfloat16, tag=f"ag2i{h}", name=f"ag2i{h}")
                      for h in range(HPC)]
            ag2_out = [dram.tile([NCORES * D_V, T], dt.bfloat16, tag=f"ag2o{h}",
                                 name=f"ag2o{h}", addr_space="Shared")
                       for h in range(HPC)]

            # ============================================================
            # Phase A: qkv^T = Wa^T @ h^T   [2112, 256] feature-major
            # ============================================================
            with contextlib.ExitStack() as pa:
                h_pool = pa.enter_context(tc.tile_pool(name="h", bufs=1))
                wa_pool = pa.enter_context(tc.tile_pool(name="wa", bufs=6))
                qkv_pool = pa.enter_context(tc.tile_pool(name="qkv", bufs=1))
                x2_pool = pa.enter_context(tc.tile_pool(name="x2", bufs=3))
                agt_pool = pa.enter_context(tc.tile_pool(name="agt", bufs=3))
                ps_a = pa.enter_context(tc.tile_pool(name="ps_a", bufs=3, space="PSUM"))
                ps_ss = pa.enter_context(tc.tile_pool(name="ps_ss", bufs=1, space="PSUM"))
                ps_bc = pa.enter_context(tc.tile_pool(name="ps_bc", bufs=1, space="PSUM"))

                h_all = h_pool.tile([128, 56, TSH], dt.bfloat16, tag="h_all", name="h_all")
                nc.sync.dma_start(h_all[:], hT.rearrange("p (a t) -> p a t", a=56))

                # f32 staging only for the kv groups (normed before AG)
                qkv = [qkv_pool.tile([128, TSH], dt.float32, tag=f"qkv{m}",
                                     name=f"qkv{m}") for m in range(4)]
                kp_raw = qkv_pool.tile([64, TSH], dt.float32, tag="kp_raw", name="kp_raw")
                kp2 = qkv_pool.tile([32, TSH], dt.float32, tag="kp2", name="kp2")

                ss_q = ps_ss.tile([1, TSH], dt.float32, tag="ssq", name="ssq")
                ss_kv = ps_ss.tile([1, TSH], dt.float32, tag="sskv", name="sskv")

                def rstd_bcast(ss, d, name):
                    # [1,T] -> scale+eps -> PE broadcast to 128 partitions ->
                    # full-width DVE reciprocal + ACT sqrt (fast wide ops)
                    ms = x2_pool.tile([1, TSH], dt.float32, tag="ms", name="ms")
                    nc.scalar.activation(ms[:], ss[:], AF.Copy, bias=EPS, scale=1.0 / d)
                    pb = ps_bc.tile([128, TSH], dt.float32, tag=f"bc{name}", name=f"bc{name}")
                    nc.tensor.matmul(pb[:], ones_f[:], ms[:], start=True, stop=True)
                    inv = x2_pool.tile([128, TSH], dt.float32, tag=f"iv{name}", name=f"iv{name}")
                    nc.vector.reciprocal(inv[:], pb[:])
                    rstd = x2_pool.tile([128, TSH], dt.float32, tag=f"rs{name}", name=f"rs{name}")
                    nc.scalar.activation(rstd[:], inv[:], AF.Sqrt)
                    return rstd

                for m in [12, 13, 14, 15, 16] + list(range(12)):
                    mp = 64 if m == 16 else 128
                    psum = ps_a.tile([128, TSH], dt.float32, tag="pa", name="pa")
                    for kc in range(7):
                        chunk = wa_pool.tile([128, 8, 128], dt.bfloat16, tag="wa_c", name="wa_c")
                        r0 = (m * 56 + kc * 8) * 128
                        nc.sync.dma_start(
                            chunk[:],
                            wa[r0:r0 + 1024, :].rearrange("(p a) f -> p a f", a=8),
                        )
                        for k8 in range(8):
                            k = kc * 8 + k8
                            nc.tensor.matmul(
                                psum[:mp, :],
                                chunk[:, k8, :mp],
                                h_all[:, k, :],
                                start=(k == 0),
                                stop=(k == 55),
                                skip_group_check=True,
                            )
                    if m < 12:
                        # q group: raw bf16 evict straight to the AG buffer
                        agt = agt_pool.tile([128, TSH], dt.bfloat16, tag="agt", name="agt")
                        nc.vector.tensor_copy(agt[:], psum[:])
                        nc.sync.dma_start(agq_in[m // 4][:, m % 4, :], agt[:])
                        x2 = x2_pool.tile([128, TSH], dt.bfloat16, tag="x2", name="x2")
                        nc.vector.tensor_mul(x2[:], agt[:], agt[:])
                        nc.tensor.matmul(
                            ss_q[:], ones_b[:], x2[:], start=(m == 0), stop=(m == 11),
                            skip_group_check=True,
                        )
                    elif m < 16:
                        nc.scalar.copy(qkv[m - 12][:], psum[:])
                        x2 = x2_pool.tile([128, TSH], dt.bfloat16, tag="x2", name="x2")
                        nc.vector.tensor_mul(x2[:], qkv[m - 12][:], qkv[m - 12][:])
                        nc.tensor.matmul(
                            ss_kv[:], ones_b[:], x2[:], start=(m == 12), stop=(m == 15),
                            skip_group_check=True,
                        )
                    else:
                        nc.scalar.copy(kp_raw[:], psum[:64, :])
                        # move the x2 half to base partition 0 for the DVE ops
                        nc.sync.dma_start(kp2[:], kp_raw[32:64, :])
                        # rope k_pe, write token-stacked [128,128], launch AG
                        kr1 = agt_pool.tile([32, TSH], dt.bfloat16, tag="kr1", name="kr1")
                        kr2 = agt_pool.tile([32, TSH], dt.bfloat16, tag="kr2", name="kr2")
                        t1 = x2_pool.tile([32, TSH], dt.bfloat16, tag="t1", name="t1")
                        t2 = x2_pool.tile([32, TSH], dt.bfloat16, tag="t2", name="t2")
                        nc.vector.tensor_mul(t1[:], kp_raw[0:32, :], csc_s[:])
                        nc.vector.tensor_mul(t2[:], kp2[:], csn_s[:])
                        nc.vector.tensor_sub(kr1[:], t1[:], t2[:])
                        t3 = x2_pool.tile([32, TSH], dt.bfloat16, tag="t1", name="t1")
                        t4 = x2_pool.tile([32, TSH], dt.bfloat16, tag="t2", name="t2")
                        nc.vector.tensor_mul(t3[:], kp_raw[0:32, :], csn_s[:])
                        nc.vector.tensor_mul(t4[:], kp2[:], csc_s[:])
                        nc.vector.tensor_add(kr2[:], t3[:], t4[:])
                        nc.sync.dma_start(agkpe_in[0:32, :], kr1[:, 0:128])
                        nc.sync.dma_start(agkpe_in[64:96, :], kr1[:, 128:256])
                        nc.sync.dma_start(agkpe_in[32:64, :], kr2[:, 0:128])
                        nc.sync.dma_start(agkpe_in[96:128, :], kr2[:, 128:256])
                        nc.gpsimd.collective_compute(
                            "AllGather", mybir.AluOpType.bypass,
                            replica_groups=rg,
                            ins=[agkpe_in.opt()], outs=[agkpe_out.opt()])
                        # kv group done: norm kv latents, launch kv AG
                        bc_kv = rstd_bcast(ss_kv, KVLR, "kv")
                        for mm in range(4):
                            agt = agt_pool.tile([128, TSH], dt.bfloat16, tag="agt", name="agt")
                            nc.vector.tensor_mul(agt[:], qkv[mm][:], bc_kv[:])
                            nc.sync.dma_start(agkv_in[:, mm, :], agt[:])
                        nc.gpsimd.collective_compute(
                            "AllGather", mybir.AluOpType.bypass,
                            replica_groups=rg,
                            ins=[agkv_in.opt()], outs=[agkv_out.opt()])
                    if m == 3 or m == 7:
                        p3 = m // 4
                        nc.gpsimd.collective_compute(
                            "AllGather", mybir.AluOpType.bypass,
                            replica_groups=rg,
                            ins=[agq_in[p3].opt()], outs=[agq_out[p3].opt()])

                # q rstd broadcast row -> part2, launch last q AG
                bc_q = rstd_bcast(ss_q, QLR, "q")
                brs = agt_pool.tile([128, TSH], dt.bfloat16, tag="brs", name="brs")
                nc.vector.tensor_copy(brs[:], bc_q[:])
                nc.sync.dma_start(agq_in[2][:, 4, :], brs[:])
                nc.gpsimd.collective_compute(
                    "AllGather", mybir.AluOpType.bypass,
                    replica_groups=rg,
                    ins=[agq_in[2].opt()], outs=[agq_out[2].opt()])

            # ============================================================
            # Phase B: Q^T, K^T (feature-major) and V (token-major)
            # ============================================================
            bpools = contextlib.ExitStack()
            with bpools:
                act = bpools.enter_context(tc.tile_pool(name="act", bufs=1))
                agq_pool = bpools.enter_context(tc.tile_pool(name="agq", bufs=3))
                agkv_pool = bpools.enter_context(tc.tile_pool(name="agkv", bufs=2))
                rp_pool = bpools.enter_context(tc.tile_pool(name="rp", bufs=4))
                pbp = contextlib.ExitStack()
                ps_q = pbp.enter_context(tc.tile_pool(name="ps_q", bufs=4, space="PSUM"))
                ps_kv = pbp.enter_context(tc.tile_pool(name="ps_kv", bufs=2, space="PSUM"))

                qt_n = [act.tile([128, T], dt.bfloat16, tag=f"qtn{h}", name=f"qtn{h}")
                        for h in range(HPC)]
                qraw = act.tile([128, T], dt.bfloat16, tag="qraw", name="qraw")
                qt_r = [act.tile([64, T], dt.bfloat16, tag=f"qtr{h}", name=f"qtr{h}")
                        for h in range(HPC)]
                kt_n = [act.tile([128, T], dt.bfloat16, tag=f"ktn{h}", name=f"ktn{h}")
                        for h in range(HPC)]
                kpe_t = act.tile([64, T], dt.bfloat16, tag="kpet", name="kpet")
                # v2_t[ti]: both heads' V for token tile ti, cols h*128..
                v2_t = [act.tile([128, 2 * D_V], dt.bfloat16, tag=f"v{i}", name=f"v{i}")
                        for i in range(16)]
                akv_all = [agkv_pool.tile([128, 4, 512], dt.bfloat16, tag=f"akv{rp}",
                                          name=f"akv{rp}") for rp in range(4)]

                # --- KV path first: only needs agkv/agkpe (land early) ---
                for rp in range(4):
                    tsl = slice(rp * 512, (rp + 1) * 512)
                    akv = akv_all[rp]
                    for s in range(2):
                        r = 2 * rp + s
                        nc.sync.dma_start(
                            akv[:, :, s * TSH:(s + 1) * TSH],
                            agkv_out[r * 128:(r + 1) * 128, :, :])
                        nc.sync.dma_start(
                            kpe_t[:, r * TSH:r * TSH + 128],
                            agkpe_out[r * 128:r * 128 + 64, :])
                        nc.sync.dma_start(
                            kpe_t[:, r * TSH + 128:(r + 1) * TSH],
                            agkpe_out[r * 128 + 64:(r + 1) * 128, :])
                    for h in range(HPC):
                        pk = ps_kv.tile([128, 512], dt.float32, tag="pkv", name="pkv")
                        for kk in range(4):
                            nc.tensor.matmul(
                                pk[:], wkvb_t[:, kk, h * 128:(h + 1) * 128],
                                akv[:, kk, :],
                                start=(kk == 0), stop=(kk == 3),
                                skip_group_check=True)
                        nc.scalar.copy(kt_n[h][:, tsl], pk[:])
                    for s4 in range(4):
                        ti = 4 * rp + s4
                        pv = ps_kv.tile([128, 2 * D_V], dt.float32, tag="pv_b", name="pv_b")
                        for kk in range(4):
                            nc.tensor.matmul(
                                pv[:],
                                akv[:, kk, s4 * 128:(s4 + 1) * 128],
                                wkvb_t[:, kk, 256:512],
                                start=(kk == 0), stop=(kk == 3),
                                skip_group_check=True)
                        nc.scalar.copy(v2_t[ti][:], pv[:])

                # --- Q path: 3 packed matmuls per (rp, kq); rstd applied
                # on eviction; rope per-rp ---
                for rp in range(4):
                    tsl = slice(rp * 512, (rp + 1) * 512)
                    aq = agq_pool.tile([128, 12, 512], dt.bfloat16, tag="aq", name="aq")
                    rstd_bc = rp_pool.tile([128, 512], dt.bfloat16, tag="rsb", name="rsb")
                    for s in range(2):
                        r = 2 * rp + s
                        ssl = slice(s * TSH, (s + 1) * TSH)
                        for p3 in range(3):
                            nc.sync.dma_start(
                                aq[:, p3 * 4:p3 * 4 + 4, ssl],
                                agq_out[p3][r * 128:(r + 1) * 128, 0:4, :])
                        nc.sync.dma_start(
                            rstd_bc[:, ssl], agq_out[2][r * 128:(r + 1) * 128, 4, :])
                    pnA = ps_q.tile([128, 512], dt.float32, tag="pq", name="pq")
                    pnB = ps_q.tile([128, 512], dt.float32, tag="pq", name="pq")
                    pR = ps_q.tile([128, 512], dt.float32, tag="pq", name="pq")
                    for kq in range(12):
                        nc.tensor.matmul(
                            pnA[:], wqb_t[:, kq, 0:128], aq[:, kq, :],
                            start=(kq == 0), stop=(kq == 11), skip_group_check=True)
                        nc.tensor.matmul(
                            pnB[:], wqb_t[:, kq, 128:256], aq[:, kq, :],
                            start=(kq == 0), stop=(kq == 11), skip_group_check=True)
                        nc.tensor.matmul(
                            pR[:], wqb_t[:, kq, 256:384], aq[:, kq, :],
                            start=(kq == 0), stop=(kq == 11), skip_group_check=True)
                    nc.vector.tensor_mul(qt_n[0][:, tsl], pnA[:], rstd_bc[:])
                    nc.vector.tensor_mul(qt_n[1][:, tsl], pnB[:], rstd_bc[:])
                    nc.vector.tensor_mul(qraw[:, tsl], pR[:], rstd_bc[:])
                    # rope: qraw rows [h0ev, h0od, h1ev, h1od] (32 each)
                    x2h0 = rp_pool.tile([32, 512], dt.bfloat16, tag="xx", name="xx")
                    x1h1 = rp_pool.tile([32, 512], dt.bfloat16, tag="xx", name="xx")
                    x2h1 = rp_pool.tile([32, 512], dt.bfloat16, tag="xx", name="xx")
                    nc.sync.dma_start(x2h0[:], qraw[32:64, tsl])
                    nc.sync.dma_start(x1h1[:], qraw[64:96, tsl])
                    nc.sync.dma_start(x2h1[:], qraw[96:128, tsl])
                    for h, (x1, x2) in enumerate([(qraw[0:32, tsl], x2h0[:]),
                                                  (x1h1[:], x2h1[:])]):
                        cc = csc_f[:, tsl]
                        ss = csn_f[:, tsl]
                        t1 = rp_pool.tile([32, 512], dt.bfloat16, tag="qt1", name="qt1")
                        t2 = rp_pool.tile([32, 512], dt.bfloat16, tag="qt2", name="qt2")
                        nc.vector.tensor_mul(t1[:], x1, cc)
                        nc.vector.tensor_mul(t2[:], x2, ss)
                        nc.vector.tensor_sub(qt_r[h][0:32, tsl], t1[:], t2[:])
                        t3 = rp_pool.tile([32, 512], dt.bfloat16, tag="qt1", name="qt1")
                        t4 = rp_pool.tile([32, 512], dt.bfloat16, tag="qt2", name="qt2")
                        nc.vector.tensor_mul(t3[:], x1, ss)
                        nc.vector.tensor_mul(t4[:], x2, cc)
                        r2t = rp_pool.tile([32, 512], dt.bfloat16, tag="r2t", name="r2t")
                        nc.vector.tensor_add(r2t[:], t3[:], t4[:])
                        nc.sync.dma_start(qt_r[h][32:64, tsl], r2t[:])

                pbp.close()

                # ========================================================
                # Phase C: attention per head
                # ========================================================
                with contextlib.ExitStack() as pc:
                    pt_pool = pc.enter_context(tc.tile_pool(name="pt", bufs=3))
                    sm_pool = pc.enter_context(tc.tile_pool(name="sm", bufs=2))
                    ps_s = pc.enter_context(tc.tile_pool(name="ps_s", bufs=3, space="PSUM"))
                    ps_pv = pc.enter_context(tc.tile_pool(name="ps_pv", bufs=2, space="PSUM"))
                    ps_l = pc.enter_context(tc.tile_pool(name="ps_l", bufs=1, space="PSUM"))
                    ps_b = pc.enter_context(tc.tile_pool(name="ps_b", bufs=1, space="PSUM"))

                    for h in range(HPC):
                        attn = act.tile([128, T], dt.bfloat16, tag=f"attn{h}", name=f"attn{h}")
                        for j in range(4):
                            qsl = slice(j * 512, (j + 1) * 512)
                            nk = 4 * j + 4
                            ppv = ps_pv.tile([128, 512], dt.float32, tag="ppv", name="ppv")
                            pl = ps_l.tile([1, 512], dt.float32, tag="pl", name="pl")
                            for ki in range(nk):
                                ksl = slice(ki * 128, (ki + 1) * 128)
                                # diagonal tiles: columns left of the diagonal
                                # are fully masked -> compute only the suffix
                                off = max(0, (ki - 4 * j) * 128)
                                w = 512 - off
                                qs0 = j * 512 + off
                                ps = ps_s.tile([128, 512], dt.float32, tag="ps", name="ps")
                                nc.tensor.matmul(
                                    ps[:, off:], kt_n[h][:, ksl],
                                    qt_n[h][:, qs0:qs0 + w],
                                    start=True, stop=False, skip_group_check=True)
                                nc.tensor.matmul(
                                    ps[:, off:], kpe_t[:, ksl],
                                    qt_r[h][:, qs0:qs0 + w],
                                    start=False, stop=True, skip_group_check=True)
                                if ki >= 4 * j:
                                    nc.vector.tensor_add(
                                        ps[:, off:off + 128], ps[:, off:off + 128],
                                        mask_sb[:])
                                pt = pt_pool.tile([128, 512], dt.bfloat16, tag="pt", name="pt")
                                nc.scalar.activation(pt[:, off:], ps[:, off:], AF.Exp)
                                nc.tensor.matmul(
                                    pl[:, off:], ones_b[:], pt[:, off:],
                                    start=(ki == 0), stop=(ki == nk - 1),
                                    skip_group_check=True)
                                nc.tensor.matmul(
                                    ppv[:, off:], v2_t[ki][:, h * D_V:(h + 1) * D_V],
                                    pt[:, off:],
                                    start=(ki == 0), stop=(ki == nk - 1),
                                    skip_group_check=True)
                            # normalize: broadcast l first, then full-width recip
                            pls = sm_pool.tile([1, 512], dt.float32, tag="pls", name="pls")
                            nc.scalar.copy(pls[:], pl[:])
                            pb = ps_b.tile([128, 512], dt.float32, tag="pb", name="pb")
                            nc.tensor.matmul(pb[:], ones_f[:], pls[:],
                                             start=True, stop=True,
                                             skip_group_check=True)
                            rb = sm_pool.tile([128, 512], dt.float32, tag="rb", name="rb")
                            nc.vector.reciprocal(rb[:], pb[:])
                            nc.vector.tensor_mul(attn[:, qsl], ppv[:], rb[:])
                        nc.sync.dma_start(ag2_in[h][:], attn[:])
                        nc.gpsimd.collective_compute(
                            "AllGather", mybir.AluOpType.bypass,
                            replica_groups=rg,
                            ins=[ag2_in[h].opt()], outs=[ag2_out[h].opt()])

            # ============================================================
            # Phase E: o_proj column slice, two K-waves (head0 then head1)
            # ============================================================
            with contextlib.ExitStack() as pe:
                ao_pool = pe.enter_context(tc.tile_pool(name="ao", bufs=1))
                oa_pool = pe.enter_context(tc.tile_pool(name="oa", bufs=1))
                oo_pool = pe.enter_context(tc.tile_pool(name="oo", bufs=3))
                ps_o = pe.enter_context(tc.tile_pool(name="ps_o", bufs=8, space="PSUM"))

                stripsA = [ao_pool.tile([128, T], dt.bfloat16, tag=f"sa{r}", name=f"sa{r}")
                           for r in range(8)]
                stripsB = [ao_pool.tile([128, T], dt.bfloat16, tag=f"sb{r}", name=f"sb{r}")
                           for r in range(8)]
                otA = [oa_pool.tile([128, T], dt.bfloat16, tag=f"oa{mt}", name=f"oa{mt}")
                       for mt in range(7)]
                for r in range(8):
                    nc.sync.dma_start(stripsA[r][:], ag2_out[0][r * 128:(r + 1) * 128, :])
                # wave A: heads 0,2,..,14 (wo blocks 2r)
                for mt in range(7):
                    msl = slice(mt * 128, (mt + 1) * 128)
                    pes = [ps_o.tile([128, 512], dt.float32, tag="po", name="po")
                           for _ in range(4)]
                    for r in range(8):
                        for n in range(4):
                            nc.tensor.matmul(
                                pes[n][:], wo_t[:, 2 * r, msl],
                                stripsA[r][:, n * 512:(n + 1) * 512],
                                start=(r == 0), stop=(r == 7),
                                skip_group_check=True)
                    for n in range(4):
                        nc.scalar.copy(otA[mt][:, n * 512:(n + 1) * 512], pes[n][:])
                for r in range(8):
                    nc.sync.dma_start(stripsB[r][:], ag2_out[1][r * 128:(r + 1) * 128, :])
                # wave B: heads 1,3,..,15 (wo blocks 2r+1), add wave A partial
                for mt in range(7):
                    msl = slice(mt * 128, (mt + 1) * 128)
                    pes = [ps_o.tile([128, 512], dt.float32, tag="po", name="po")
                           for _ in range(4)]
                    for r in range(8):
                        for n in range(4):
                            nc.tensor.matmul(
                                pes[n][:], wo_t[:, 2 * r + 1, msl],
                                stripsB[r][:, n * 512:(n + 1) * 512],
                                start=(r == 0), stop=(r == 7),
                                skip_group_check=True)
                    ot = oo_pool.tile([128, T], dt.float32, tag="ot", name="ot")
                    for n in range(4):
                        nc.vector.tensor_add(
                            ot[:, n * 512:(n + 1) * 512], pes[n][:],
                            otA[mt][:, n * 512:(n + 1) * 512])
                    nc.sync.dma_start(out[msl, :], ot[:])


def _prep_inputs(hidden_states, positions, W_qkv_a, gamma_q, W_qb, gamma_kv,
                 W_kvb, W_o):
    f32 = np.float32
    perm = np.concatenate([np.arange(0, D_ROPE, 2), np.arange(1, D_ROPE, 2)])
    scale = np.float32(D_QK ** -0.5)

    def pmajor(w, nblk):
        # [nblk*128, F] -> [128, nblk*F] so a partition line is contiguous
        F = w.shape[1]
        return np.ascontiguousarray(
            w.reshape(nblk, 128, F).transpose(1, 0, 2).reshape(128, nblk * F)
        ).astype(BF16)

    # A-projection weights: de-interleave k_pe output cols, block layout
    Wa = np.asarray(W_qkv_a, f32).copy()
    Wa[:, QLR + KVLR:] = Wa[:, QLR + KVLR:][:, perm]
    Wa = np.concatenate([Wa, np.zeros((H, 64), f32)], axis=1)  # pad 2112->2176
    wa_b = (
        Wa.reshape(7, 8, 128, 17, 128)   # [kc, k8, p, m, f]
        .transpose(3, 0, 2, 1, 4)        # [m, kc, p, k8, f]
        .reshape(17 * 56 * 128, 128)
        .astype(BF16)
    )

    # q_b weights: fold gamma_q and score scale, de-interleave rope cols,
    # pack per-core as [h0 nope | h1 nope | h0 ev | h0 od | h1 ev | h1 od]
    Wqb = (np.asarray(W_qb, f32) * np.asarray(gamma_q, f32)[:, None] * scale)
    Wqb = Wqb.reshape(QLR, NH, D_QK)
    Wqb_n = Wqb[:, :, :D_NOPE]
    Wqb_r = Wqb[:, :, D_NOPE:][:, :, perm]   # [QLR, NH, 64] ev|od

    # kv_b weights: fold gamma_kv
    Wkvb = (np.asarray(W_kvb, f32) * np.asarray(gamma_kv, f32)[:, None])
    Wkvb = Wkvb.reshape(KVLR, NH, D_NOPE + D_V)

    Wo = np.asarray(W_o, f32)

    hTf = np.asarray(hidden_states, f32).T.astype(BF16)  # [H, T]

    pos = np.asarray(positions, f32)
    inv_freq = 1.0 / (THETA ** (np.arange(D_ROPE // 2, dtype=f32) / (D_ROPE // 2)))
    freqs = pos[:, None] * inv_freq[None, :]          # [T, 32]
    cos = np.cos(freqs).astype(f32).T                 # [32, T]
    sin = np.sin(freqs).astype(f32).T
    cs = np.concatenate([cos, sin], axis=0)           # [64, T]

    kk = np.arange(128)[:, None]
    qq = np.arange(128)[None, :]
    mask1 = np.where(qq < kk, np.float32(NEG), np.float32(0.0)).astype(f32)

    in_maps = []
    for c in range(NCORES):
        h0, h1 = 2 * c, 2 * c + 1
        wqb_c = np.concatenate(
            [Wqb_n[:, h0, :], Wqb_n[:, h1, :], Wqb_r[:, h0, :], Wqb_r[:, h1, :]],
            axis=1)                                  # [QLR, 384]
        wkvb_c = np.concatenate(
            [Wkvb[:, h0, :D_NOPE], Wkvb[:, h1, :D_NOPE],
             Wkvb[:, h0, D_NOPE:], Wkvb[:, h1, D_NOPE:]], axis=1)  # [KVLR, 512]
        hsh = np.ascontiguousarray(hTf[:, c * TSH:(c + 1) * TSH])  # [H, TSH]
        hsh = hsh.reshape(56, 128, TSH).transpose(1, 0, 2).reshape(128, 56 * TSH)
        in_maps.append({
            "hT": np.ascontiguousarray(hsh),
            "wa": wa_b,
            "wqb": pmajor(wqb_c, 12),
            "wkvb": pmajor(wkvb_c, 4),
            "wo": pmajor(Wo[:, c * WO_COLS:(c + 1) * WO_COLS], 16),
            "cs_sh": np.ascontiguousarray(cs[:, c * TSH:(c + 1) * TSH]),
            "cs_full": cs,
            "mask1": mask1,
        })
    return in_maps


def kernel(hidden_states, positions, W_qkv_a, gamma_q, W_qb, gamma_kv, W_kvb,
           W_o, _trace=False, _tmpdir=None):
    from concourse.bass_utils import run_bass_kernel_spmd

    if "nc" not in _CACHE:
        _CACHE["nc"] = _build()
    nc = _CACHE["nc"]

    in_maps = _prep_inputs(hidden_states, positions, W_qkv_a, gamma_q, W_qb,
                           gamma_kv, W_kvb, W_o)
    res = run_bass_kernel_spmd(nc, in_maps, list(range(NCORES)), trace=_trace,
                               tmpdir=_tmpdir)
    _CACHE["last_result"] = res
    out = np.concatenate(
        [res.results[c]["out"].T for c in range(NCORES)], axis=1)
    return out.astype(np.float32)
